# revision 1
# baseline (speedup 1.0000x reference)
"""GAT message-passing kernel for Trainium2, 8 NeuronCores (graph-parallel).

Contract: kernel(**inputs) takes FULL inputs (x [50000,128] f32,
edge_index [2,800000] i32, weights/biases) and returns the FULL output
[50000, 128] f32. Self-contained: preprocessing (numpy) + Bass program +
PJRT exec are all in this file.

Sharding / algorithm (per core, destinations sharded 6250/core):
- Host: add self-loops; LPT-pack each core's destinations into 49 blocks of
  <=128 so per-(block, half) edge counts are balanced; bucket+sort edges by
  (block, source-half); emit int16 gather indices (wrapped [16 x n/16],
  replicated across the 8 Q7 cores) and per-chunk block-local dest ids.
- Phase 1 (dense, redundant on every core): K=relu(x@Wk+kb), V=x@W from a
  host-pretransposed fp16 xT via one 192-col matmul per 128-node tile,
  packed into two half-tables (25001 rows each, int16-indexable, + a zeros
  row for padding) of 512B rows [K as f32 | V as bf16]; Q=relu(x@Wq+qb) for
  local nodes only, SBUF-resident.
- Phase 2 (attention, per 128-dest block): dma_gather the block's edge
  sources (2 gathers, one per half-table); per 128-edge chunk build the
  one-hot OH[e,d] with a DVE is_equal against an iota (chunk-minor layout to
  hit the 2x DVE mode; pad edges carry dest=-1 so their one-hot rows are
  zero -> self-masking), PE-transpose it, expand Q to edges with one matmul,
  score = per-head reduce of Q*K (K read back as f32), exp on ACT (bf16),
  scale V by exp, then a single PSUM-accumulated matmul per chunk computes
  both sum(exp*V) and sum(exp) (concatenated rhs). Normalize + bias at block
  end; host inverse-permutes the balanced block layout.
Softmax max-subtraction is dropped (scores ~O(30) max, exp stays in fp32
range; matches the reference exactly up to rounding).
"""
import math
import os

import numpy as np

import ml_dtypes

import concourse.bass as bass
import concourse.mybir as mybir
import concourse.tile as tile
from concourse import bacc

P = 128
C = 8                    # cores
N, F, E = 50000, 128, 800000
H, A, U = 8, 64, 128     # heads, att units, units
HD = A // H              # per-head q/k dim (8)
UD = U // H              # per-head v dim (16)
NPC = N // C             # nodes per core
NB = math.ceil(NPC / P)  # dest blocks per core (49)
HALF = N // 2            # table split point (fits int16 indices)
BF16 = mybir.dt.bfloat16
F32 = mybir.dt.float32
NP_BF16 = ml_dtypes.bfloat16
F16 = mybir.dt.float16
NP_F16 = np.float16

KVROW = 256              # bf16 elems per packed row: [K as f32-bitcast (128) | V bf16 (128)] = 512B


# ---------------------------------------------------------------- preprocessing
def preprocess(x, edge_index, query_kernel, query_bias, key_kernel, key_bias,
               kernel, bias):
    """Build per-core input maps + the uniform structure params.

    Destinations are assigned to (core, block) with an LPT greedy pack so
    per-block edge counts are balanced -> minimal chunk padding. Returns
    (in_maps, CHH, perm) where perm[c, b*P+i] is the global node id stored
    at output row (c, b*P+i), or -1 for unused slots.
    """
    import heapq
    x = np.asarray(x, np.float32)
    ei = np.asarray(edge_index, np.int64)
    row = np.concatenate([ei[0], np.arange(N, dtype=np.int64)])   # dest
    col = np.concatenate([ei[1], np.arange(N, dtype=np.int64)])   # src
    Et = row.shape[0]

    # per-core source relabeling: core c stores node n's KV row at
    # (n - off_c) mod N with off_c = c*NPC - (HALF - NPC//2), so each core's
    # self-loop sources straddle the KV0/KV1 split -> halves stay balanced.
    offs = np.array([c * NPC - (HALF - NPC // 2) for c in range(C)])
    core_e = row // NPC
    srow = (col - offs[core_e]) % N
    half_e = (srow >= HALF).astype(np.int64)
    deg0 = np.bincount(row[half_e == 0], minlength=N)
    deg1 = np.bincount(row[half_e == 1], minlength=N)
    deg = deg0 + deg1
    # --- balanced block assignment per core: greedy pack minimizing the max
    # per-(block, half) load (that max sets CHH = the gather chunk count) ---
    blk_of = np.empty(N, np.int32)
    loc_of = np.empty(N, np.int32)
    perm = np.full((C, NB * P), -1, np.int64)
    for c in range(C):
        nodes = np.arange(c * NPC, (c + 1) * NPC)
        nodes = nodes[np.argsort(-deg[nodes], kind="stable")]
        l0 = np.zeros(NB, np.int64)
        l1 = np.zeros(NB, np.int64)
        cnt = np.zeros(NB, np.int64)
        for n in nodes:
            cost = np.maximum(l0 + deg0[n], l1 + deg1[n])
            cost[cnt >= P] = 1 << 60
            b = int(np.argmin(cost))
            blk_of[n] = b
            loc_of[n] = cnt[b]
            perm[c, b * P + cnt[b]] = n
            l0[b] += deg0[n]
            l1[b] += deg1[n]
            cnt[b] += 1

    core = core_e
    lb = blk_of[row].astype(np.int64)
    ld = loc_of[row].astype(np.int64)
    half = half_e

    grp = (core * NB + lb) * 2 + half                  # [Et] in [0, C*NB*2)
    order = np.argsort(grp * 128 + ld, kind="stable")  # group, dest-sorted inside
    gs = grp[order]
    counts = np.bincount(grp, minlength=C * NB * 2)
    CHH = max(1, int(math.ceil(counts.max() / P)))     # chunks per half-gather
    SPH = CHH * P                                      # slots per half
    starts = np.zeros(C * NB * 2, np.int64)
    starts[1:] = np.cumsum(counts)[:-1]
    pos = np.arange(Et) - starts[gs]
    slot = gs * SPH + pos

    idx_all = np.full(C * NB * 2 * SPH, HALF, np.int16)   # pad -> zeros row
    idx_all[slot] = (srow - half * HALF)[order].astype(np.int16)
    dest_all = np.full(C * NB * 2 * SPH, -1.0, np.float32)
    dest_all[slot] = ld[order].astype(np.float32)

    idx_all = idx_all.reshape(C, NB * 2, CHH * 8, 16)
    idxg = np.tile(idx_all.transpose(0, 3, 1, 2).reshape(C, 16, NB * 2 * CHH * 8),
                   (1, 8, 1))                              # [C, 128, NB*2*CHH*8]
    destc = dest_all.reshape(C, NB * 2, CHH, P).transpose(0, 3, 1, 2) \
                    .reshape(C, P, NB * 2 * CHH)           # [C, 128, NB*2*CHH]

    xT = np.ascontiguousarray(x.T.astype(NP_F16))          # [128, N] fp16
    xq = np.zeros((C, P, NB * P), NP_F16)
    for c in range(C):
        valid = perm[c] >= 0
        xq[c][:, valid] = xT[:, perm[c][valid]]

    wcat = np.concatenate(
        [np.asarray(query_kernel), np.asarray(key_kernel), np.asarray(kernel)],
        axis=1).astype(NP_F16)                             # [128, 256] fp16
    qkb = np.tile(np.concatenate([np.asarray(query_bias), np.asarray(key_bias)])
                  .astype(np.float32)[None, :], (P, 1))    # [128, 128]
    outb = np.tile(np.asarray(bias, np.float32)[None, :], (P, 1))
    nobias = bool(np.all(qkb == 0.0) and np.all(outb == 0.0))

    in_maps = []
    for c in range(C):
        in_maps.append({
            "xT": np.roll(xT, -int(offs[c]), axis=1),   # table row r = node (r+off_c)%N
            "xq": xq[c], "wcat": wcat, "qkb": qkb, "outb": outb,
            "idxg": np.ascontiguousarray(idxg[c]),
            "destc": np.ascontiguousarray(destc[c]),
        })
    return in_maps, CHH, perm, nobias


# ---------------------------------------------------------------- bass program
def build_program(CHH, reps=1, nobias=False):
    nc = bacc.Bacc(None, target_bir_lowering=False, debug=False)

    xT = nc.dram_tensor("xT", [P, N], F16, kind="ExternalInput")
    xq = nc.dram_tensor("xq", [P, NB * P], F16, kind="ExternalInput")
    wcat_d = nc.dram_tensor("wcat", [P, 256], F16, kind="ExternalInput")
    qkb_d = nc.dram_tensor("qkb", [P, P], F32, kind="ExternalInput")
    outb_d = nc.dram_tensor("outb", [P, P], F32, kind="ExternalInput")
    idxg_d = nc.dram_tensor("idxg", [P, NB * 2 * CHH * 8], mybir.dt.int16,
                            kind="ExternalInput")
    destc_d = nc.dram_tensor("destc", [P, NB * 2 * CHH], F32, kind="ExternalInput")
    out_d = nc.dram_tensor("out", [NB * P, U], F32, kind="ExternalOutput")

    KV0 = nc.dram_tensor("KV0", [HALF + 1, KVROW], BF16)
    KV1 = nc.dram_tensor("KV1", [N - HALF + 1, KVROW], BF16)

    def store_rows(r0, nr, sb_ap):
        """Store [nr, KVROW] bf16 rows starting at global row r0 into KV0/KV1."""
        if r0 >= HALF:
            nc.sync.dma_start(KV1[r0 - HALF: r0 - HALF + nr], sb_ap)
        elif r0 + nr <= HALF:
            nc.sync.dma_start(KV0[r0: r0 + nr], sb_ap)
        else:
            k = HALF - r0
            nc.sync.dma_start(KV0[r0:HALF], sb_ap[0:k])
            nc.sync.dma_start(KV1[0: nr - k], sb_ap[k:nr])

    with tile.TileContext(nc) as tc:
        with tc.tile_pool(name="const", bufs=1) as cpool:
            # resident tensors
            wcat = cpool.tile([P, 256], F16, tag="wcat")
            qkb = cpool.tile([P, P], F32, tag="qkb")
            outb = cpool.tile([P, P], F32, tag="outb")
            idxg = cpool.tile([P, NB * 2 * CHH * 8], mybir.dt.int16, tag="idxg")
            destc = cpool.tile([P, NB * 2 * CHH], F32, tag="destc")
            qall = cpool.tile([P, NB, A], F16, tag="qall")
            iota_f = cpool.tile([P, P], F32, tag="iotaf")
            iota_b = cpool.tile([P, P], BF16, tag="iotab")
            iota_q = cpool.tile([P, P, 4], BF16, tag="iotaq")
            iota_qi = cpool.tile([P, P, 4], mybir.dt.int32, tag="iotaqi")
            destc_b = cpool.tile([P, NB * 2 * CHH], BF16, tag="destcb")
            ident = cpool.tile([P, P], BF16, tag="ident")
            iota_i = cpool.tile([P, P], mybir.dt.int32, tag="iotai")
            zrow = cpool.tile([1, KVROW], BF16, tag="zrow")

            nc.sync.dma_start(wcat[:], wcat_d[:])
            nc.sync.dma_start(qkb[:], qkb_d[:])
            nc.sync.dma_start(outb[:], outb_d[:])
            nc.sync.dma_start(idxg[:], idxg_d[:])
            nc.sync.dma_start(destc[:], destc_d[:])
            nc.gpsimd.iota(iota_i[:], pattern=[[1, P]], base=0, channel_multiplier=0)
            nc.vector.tensor_copy(iota_f[:], iota_i[:])
            nc.vector.tensor_copy(iota_b[:], iota_i[:])
            nc.gpsimd.iota(iota_qi[:], pattern=[[1, P], [0, 4]], base=0,
                           channel_multiplier=0)
            nc.vector.tensor_copy(iota_q[:], iota_qi[:])
            nc.vector.tensor_copy(destc_b[:], destc[:])
            from concourse.masks import make_identity
            make_identity(nc, ident[:])
            nc.vector.memset(zrow[:], 0.0)
            store_rows(HALF, 1, zrow[:])      # zeros row of KV0 (row HALF==25000)
            nc.sync.dma_start(KV1[N - HALF], zrow[:])  # zeros row of KV1

            for rep in range(reps):
                _emit_pipeline(nc, tc, CHH, xT, xq, out_d, KV0, KV1, store_rows,
                               wcat, qkb, outb, idxg, destc_b, qall, iota_q, ident,
                               rep, nobias)

    nc.compile()
    return nc


def _emit_pipeline(nc, tc, CHH, xT, xq, out_d, KV0, KV1, store_rows,
                   wcat, qkb, outb, idxg, destc, qall, iota_f, ident, rep,
                   nobias=False):
    r = f"r{rep}"
    # ---------------- phase 1a: Q for local nodes -> qall (f32)
    with (
        tc.tile_pool(name=f"qx{r}", bufs=3) as qxp,
        tc.tile_pool(name=f"qps{r}", bufs=2, space="PSUM") as qpsp,
        tc.tile_pool(name=f"qtmp{r}", bufs=3) as qtp,
    ):
        for b in range(NB):
            xqt = qxp.tile([P, P], F16, tag="xqt")
            nc.sync.dma_start(xqt[:], xq[:, b * P:(b + 1) * P])
            qps = qpsp.tile([P, A], F32, tag="qps")
            nc.tensor.matmul(qps[:], xqt[:], wcat[:, 0:A],
                             start=True, stop=True)
            if nobias:
                nc.scalar.activation(qall[:, b, :], qps[:],
                                     mybir.ActivationFunctionType.Relu)
            else:
                qtmp = qtp.tile([P, A], F32, tag="qtmp")
                nc.vector.tensor_tensor(out=qtmp[:], in0=qps[:],
                                        in1=qkb[:, 0:A],
                                        op=mybir.AluOpType.add)
                nc.scalar.activation(qall[:, b, :], qtmp[:],
                                     mybir.ActivationFunctionType.Relu)

    # ---------------- phase 1b: full KV table (K f32-bitcast | V bf16)
    # per 512-node macro tile: 1 x-load, 4 matmuls into 2 paired psum tiles,
    # fused DVE/ACT epilogues over 4 subtiles, 1 batched store.
    XW = 512
    NT = math.ceil(N / XW)
    with (
        tc.tile_pool(name=f"xload{r}", bufs=3) as xlp,
        tc.tile_pool(name=f"kvps{r}", bufs=4, space="PSUM") as kvpsp,
        tc.tile_pool(name=f"kvsb{r}", bufs=3) as kvsbp,
        tc.tile_pool(name=f"ktmp{r}", bufs=3) as ktp,
    ):
        for t in range(NT):
            n0 = t * XW
            nn = min(XW, N - n0)
            ns = math.ceil(nn / P)          # subtiles (4, last tile 3)
            xt = xlp.tile([P, XW], F16, tag="xt")
            nc.sync.dma_start(xt[:, 0:nn], xT[:, n0:n0 + nn])
            kvsb = kvsbp.tile([P, 4, KVROW], BF16, tag="kvsb")
            ktmp = ktp.tile([P, 4, A], F32, tag="ktmp")
            pss = []
            for pair in range(math.ceil(ns / 2)):
                ps = kvpsp.tile([P, 2, 192], F32, tag="kvps")
                pss.append(ps)
                for j in range(min(2, ns - 2 * pair)):
                    s = 2 * pair + j
                    nr = min(P, nn - s * P)
                    nc.tensor.matmul(ps[0:nr, j, :], xt[:, s * P: s * P + nr],
                                     wcat[:, A:256], start=True, stop=True)
            for pair in range(math.ceil(ns / 2)):
                np_ = min(2, ns - 2 * pair)
                ps = pss[pair]
                sl = slice(2 * pair, 2 * pair + np_)
                if nobias:
                    nc.vector.tensor_scalar_max(
                        kvsb[:, sl, 0:U].bitcast(F32), ps[:, 0:np_, 0:A], 0.0)
                else:
                    nc.vector.tensor_tensor(
                        out=ktmp[:, sl, :], in0=ps[:, 0:np_, 0:A],
                        in1=qkb[:, None, A:P].broadcast_to([P, np_, A]),
                        op=mybir.AluOpType.add)
                nc.vector.tensor_copy(kvsb[:, sl, U:KVROW], ps[:, 0:np_, A:192])
            if not nobias:
                nc.scalar.activation(kvsb[:, 0:ns, 0:U].bitcast(F32),
                                     ktmp[:, 0:ns, :],
                                     mybir.ActivationFunctionType.Relu)
            # batched store of [P, ns, KVROW]: rows n0 + s*128 + p
            full = nn == ns * P
            lo, hi = n0, n0 + nn
            if hi <= HALF or lo >= HALF:
                dst, off = (KV0, 0) if hi <= HALF else (KV1, HALF)
                if full:
                    nc.sync.dma_start(
                        dst[lo - off: hi - off].rearrange("(s p) e -> p s e", p=P),
                        kvsb[:, 0:ns, :])
                else:
                    nfull = nn // P
                    if nfull:
                        nc.sync.dma_start(
                            dst[lo - off: lo - off + nfull * P]
                            .rearrange("(s p) e -> p s e", p=P),
                            kvsb[:, 0:nfull, :])
                    rem = nn - nfull * P
                    nc.sync.dma_start(
                        dst[lo - off + nfull * P: lo - off + nn],
                        kvsb[0:rem, nfull, :])
            else:
                # crosses the HALF split: store per subtile
                for s in range(ns):
                    r0 = n0 + s * P
                    nr = min(P, N - r0)
                    store_rows(r0, nr, kvsb[0:nr, s, :])

    # ---------------- phase 2: attention per dest block, quad-batched chunks
    NCH = 2 * CHH
    NQ = math.ceil(NCH / 4)
    with (
        tc.tile_pool(name=f"kvt{r}", bufs=3) as kvtp,
        tc.tile_pool(name=f"oh{r}", bufs=6) as ohp,
        tc.tile_pool(name=f"ohtps{r}", bufs=3, space="PSUM") as ohtpsp,
        tc.tile_pool(name=f"oht{r}", bufs=6) as ohtp,
        tc.tile_pool(name=f"qeps{r}", bufs=3, space="PSUM") as qepsp,
        tc.tile_pool(name=f"prod{r}", bufs=6) as prp,
        tc.tile_pool(name=f"score{r}", bufs=6) as scp,
        tc.tile_pool(name=f"wt{r}", bufs=6) as wtp,
        tc.tile_pool(name=f"ops{r}", bufs=2, space="PSUM") as opsp,
        tc.tile_pool(name=f"fin{r}", bufs=3) as finp,
    ):
        for b in range(NB):
            kvt = kvtp.tile([P, NCH, KVROW], BF16, tag="kvt")
            for hf in range(2):
                srct = KV0 if hf == 0 else KV1
                i0 = (b * 2 + hf) * CHH * 8
                nc.gpsimd.dma_gather(
                    kvt[:, hf * CHH:(hf + 1) * CHH, :], srct[:],
                    idxg[:, i0: i0 + CHH * 8],
                    num_idxs=CHH * P, num_idxs_reg=CHH * P,
                    elem_size=KVROW, single_packet=False,
                )
            ops = opsp.tile([P, U + H], F32, tag="ops")
            for q in range(NQ):
                qn = min(4, NCH - 4 * q)
                c0 = 4 * q
                ch_base = c0
                g0 = b * NCH + c0
                oh4 = ohp.tile([P, P, 4], BF16, tag="oh4")
                nc.vector.tensor_tensor(
                    out=oh4[:, :, 0:qn],
                    in0=destc[:, g0:g0 + qn][:, None, :].broadcast_to([P, P, qn]),
                    in1=iota_f[:, :, 0:qn],
                    op=mybir.AluOpType.is_equal)
                ohtps4 = ohtpsp.tile([P, 4, P], BF16, tag="ohtps4")
                for j in range(qn):
                    nc.tensor.transpose(ohtps4[:, j, :], oh4[:, :, j], ident[:])
                oht4 = ohtp.tile([P, 4, P], F16, tag="oht4")
                nc.scalar.copy(oht4[:, 0:qn, :], ohtps4[:, 0:qn, :])
                qeps4 = qepsp.tile([P, 4, A], F32, tag="qeps4")
                for j in range(qn):
                    nc.tensor.matmul(qeps4[:, j, :], oht4[:, j, :], qall[:, b, :],
                                     start=True, stop=True)
                prod4 = prp.tile([P, 4, A], F32, tag="prod4")
                nc.vector.tensor_tensor(
                    out=prod4[:, 0:qn, :], in0=qeps4[:, 0:qn, :],
                    in1=kvt[:, c0:c0 + qn, 0:U].bitcast(F32),
                    op=mybir.AluOpType.mult)
                score4 = scp.tile([P, 4, H], F32, tag="score4")
                nc.vector.tensor_reduce(
                    score4[:, 0:qn, :],
                    prod4[:, 0:qn, :].rearrange("p q (h d) -> p q h d", h=H),
                    axis=mybir.AxisListType.X, op=mybir.AluOpType.add)
                wt4 = wtp.tile([P, 4, U + H], BF16, tag="wt4")
                nc.scalar.activation(wt4[:, 0:qn, U:U + H], score4[:, 0:qn, :],
                                     mybir.ActivationFunctionType.Exp)
                nc.vector.tensor_tensor(
                    out=wt4[:, 0:qn, 0:U].rearrange("p q (h u) -> p q h u", h=H),
                    in0=kvt[:, c0:c0 + qn, U:KVROW]
                        .rearrange("p q (h u) -> p q h u", h=H),
                    in1=wt4[:, 0:qn, U:U + H][:, :, :, None]
                        .broadcast_to([P, qn, H, UD]),
                    op=mybir.AluOpType.mult)
                for j in range(qn):
                    ch = ch_base + j
                    nc.tensor.matmul(ops[:], oh4[:, :, j], wt4[:, j, :],
                                     start=(ch == 0), stop=(ch == NCH - 1))
            recip = finp.tile([P, H], F32, tag="recip")
            nc.vector.reciprocal(recip[:], ops[:, U:U + H])
            o1 = finp.tile([P, U], F32, tag="o1")
            nc.vector.tensor_tensor(
                out=o1[:].rearrange("p (h u) -> p h u", h=H),
                in0=ops[:, 0:U].rearrange("p (h u) -> p h u", h=H),
                in1=recip[:][:, :, None].broadcast_to([P, H, UD]),
                op=mybir.AluOpType.mult)
            if not nobias:
                nc.vector.tensor_tensor(out=o1[:], in0=o1[:], in1=outb[:],
                                        op=mybir.AluOpType.add)
            nc.sync.dma_start(out_d[b * P: (b + 1) * P], o1[:])


# ---------------------------------------------------------------- execution
class SpmdRunner:
    def __init__(self, nc, n_cores=C):
        import jax
        from jax.sharding import Mesh, PartitionSpec
        from jax.experimental.shard_map import shard_map
        from concourse.bass2jax import (_bass_exec_p, install_neuronx_cc_hook,
                                        partition_id_tensor)
        install_neuronx_cc_hook()
        self.jax = jax
        self.nc = nc
        self.n_cores = n_cores
        partition_name = nc.partition_id_tensor.name if nc.partition_id_tensor else None
        in_names, out_names, out_avals = [], [], []
        for alloc in nc.m.functions[0].allocations:
            if not isinstance(alloc, mybir.MemoryLocationSet):
                continue
            name = alloc.memorylocations[0].name
            if alloc.kind == "ExternalInput":
                if name != partition_name:
                    in_names.append(name)
            elif alloc.kind == "ExternalOutput":
                out_names.append(name)
                out_avals.append(jax.core.ShapedArray(
                    tuple(alloc.tensor_shape), mybir.dt.np(alloc.dtype)))
        self.in_names, self.out_names, self.out_avals = in_names, out_names, out_avals
        n_params = len(in_names)

        all_in_names = list(in_names) + list(out_names)
        if partition_name is not None:
            all_in_names.append(partition_name)

        def _body(*args):
            operands = list(args)
            if partition_name is not None:
                operands.append(partition_id_tensor())
            outs = _bass_exec_p.bind(
                *operands,
                out_avals=tuple(out_avals),
                in_names=tuple(all_in_names),
                out_names=tuple(out_names),
                lowering_input_output_aliases=(),
                sim_require_finite=False,
                sim_require_nnan=False,
                nc=nc,
            )
            return tuple(outs)

        devices = jax.devices()[:n_cores]
        self.mesh = Mesh(np.asarray(devices), ("core",))
        n_extra = len(out_names)
        in_specs = (PartitionSpec("core"),) * (n_params + n_extra)
        out_specs = (PartitionSpec("core"),) * len(out_names)
        self.fn = jax.jit(
            shard_map(_body, mesh=self.mesh, in_specs=in_specs,
                      out_specs=out_specs, check_rep=False),
            keep_unused=True,
        )

    def put_inputs(self, in_maps):
        from jax.sharding import NamedSharding, PartitionSpec
        sharding = NamedSharding(self.mesh, PartitionSpec("core"))
        args = []
        for name in self.in_names:
            concat = np.concatenate([np.asarray(m[name]) for m in in_maps], axis=0)
            args.append(self.jax.device_put(concat, sharding))
        for av in self.out_avals:
            args.append(self.jax.device_put(
                np.zeros((self.n_cores * av.shape[0], *av.shape[1:]), av.dtype),
                sharding))
        return args

    def __call__(self, args):
        outs = self.fn(*args)
        self.jax.block_until_ready(outs)
        return outs

    def run_to_numpy(self, args):
        outs = self(args)
        res = []
        for c in range(self.n_cores):
            d = {}
            for i, name in enumerate(self.out_names):
                d[name] = np.asarray(outs[i]).reshape(
                    self.n_cores, *self.out_avals[i].shape)[c]
            res.append(d)
        return res


_CACHE = {}


def _get_runner(CHH, reps=1, nobias=False):
    key = (CHH, reps, nobias)
    if key not in _CACHE:
        nc = build_program(CHH, reps=reps, nobias=nobias)
        _CACHE[key] = SpmdRunner(nc)
    return _CACHE[key]


def kernel(x, edge_index, query_kernel, query_bias, key_kernel, key_bias,
           kernel, bias):
    in_maps, CHH, perm, nobias = preprocess(x, edge_index, query_kernel,
                                            query_bias, key_kernel, key_bias,
                                            kernel, bias)
    runner = _get_runner(CHH, nobias=nobias)
    args = runner.put_inputs(in_maps)
    res = runner.run_to_numpy(args)
    out = np.empty((N, U), np.float32)
    for c in range(C):
        valid = perm[c] >= 0
        out[perm[c][valid]] = res[c]["out"][valid]
    return out



# revision 9
# speedup vs baseline: 1.8895x; 1.8895x over previous
"""GAT message-passing kernel for Trainium2, 8 NeuronCores (graph-parallel).

Contract: kernel(**inputs) takes FULL inputs (x [50000,128] f32,
edge_index [2,800000] i32, weights/biases) and returns the FULL output
[50000, 128] f32. Self-contained: preprocessing (numpy) + Bass program +
PJRT exec are all in this file.

Sharding / algorithm (per core, destinations sharded 6250/core):
- Host: add self-loops; LPT-pack each core's destinations into 49 blocks of
  <=128 so per-(block, half) edge counts are balanced; bucket+sort edges by
  (block, source-half); emit int16 gather indices (wrapped [16 x n/16],
  replicated across the 8 Q7 cores) and per-chunk block-local dest ids.
- Phase 1 (dense, redundant on every core): K=relu(x@Wk+kb), V=x@W from a
  host-pretransposed fp16 xT via one 192-col matmul per 128-node tile,
  packed into two half-tables (25001 rows each, int16-indexable, + a zeros
  row for padding) of 512B rows [K as f32 | V as bf16]; Q=relu(x@Wq+qb) for
  local nodes only, SBUF-resident.
- Phase 2 (attention, per 128-dest block): dma_gather the block's edge
  sources (2 gathers, one per half-table); per 128-edge chunk build the
  one-hot OH[e,d] with a DVE is_equal against an iota (chunk-minor layout to
  hit the 2x DVE mode; pad edges carry dest=-1 so their one-hot rows are
  zero -> self-masking), PE-transpose it, expand Q to edges with one matmul,
  score = per-head reduce of Q*K (K read back as f32), exp on ACT (bf16),
  scale V by exp, then a single PSUM-accumulated matmul per chunk computes
  both sum(exp*V) and sum(exp) (concatenated rhs). Normalize + bias at block
  end; host inverse-permutes the balanced block layout.
Softmax max-subtraction is dropped (scores ~O(30) max, exp stays in fp32
range; matches the reference exactly up to rounding).
"""
import math
import os

import numpy as np

import ml_dtypes

import concourse.bass as bass
import concourse.mybir as mybir
import concourse.tile as tile
from concourse import bacc

P = 128
C = 8                    # cores
N, F, E = 50000, 128, 800000
H, A, U = 8, 64, 128     # heads, att units, units
HD = A // H              # per-head q/k dim (8)
UD = U // H              # per-head v dim (16)
NPC = N // C             # nodes per core
NB = math.ceil(NPC / P)  # dest blocks per core (49)
HALF = N // 2            # table split point (fits int16 indices)
BF16 = mybir.dt.bfloat16
F32 = mybir.dt.float32
NP_BF16 = ml_dtypes.bfloat16
F16 = mybir.dt.float16
NP_F16 = np.float16

KVROW = 256              # bf16 elems per packed row: [K as f32-bitcast (128) | V bf16 (128)] = 512B
GS = 5                   # chunks per attention instruction group


# ---------------------------------------------------------------- preprocessing
def preprocess(x, edge_index, query_kernel, query_bias, key_kernel, key_bias,
               kernel, bias):
    """Build per-core input maps + the uniform structure params.

    Destinations are assigned to (core, block) with an LPT greedy pack so
    per-block edge counts are balanced -> minimal chunk padding. Returns
    (in_maps, CHH, perm) where perm[c, b*P+i] is the global node id stored
    at output row (c, b*P+i), or -1 for unused slots.
    """
    import heapq
    x = np.asarray(x, np.float32)
    ei = np.asarray(edge_index, np.int64)
    row = np.concatenate([ei[0], np.arange(N, dtype=np.int64)])   # dest
    col = np.concatenate([ei[1], np.arange(N, dtype=np.int64)])   # src
    Et = row.shape[0]

    # per-core source relabeling: core c stores node n's KV row at
    # (n - off_c) mod N with off_c = c*NPC - (HALF - NPC//2), so each core's
    # self-loop sources straddle the KV0/KV1 split -> halves stay balanced.
    offs = np.array([c * NPC - (HALF - NPC // 2) for c in range(C)])
    core_e = row // NPC
    srow = (col - offs[core_e]) % N
    half_e = (srow >= HALF).astype(np.int64)
    deg0 = np.bincount(row[half_e == 0], minlength=N)
    deg1 = np.bincount(row[half_e == 1], minlength=N)
    deg = deg0 + deg1
    # --- balanced block assignment per core: greedy pack minimizing the max
    # per-(block, half) load (that max sets CHH = the gather chunk count) ---
    blk_of = np.empty(N, np.int32)
    loc_of = np.empty(N, np.int32)
    perm = np.full((C, NB * P), -1, np.int64)
    for c in range(C):
        nodes = np.arange(c * NPC, (c + 1) * NPC)
        nodes = nodes[np.argsort(-deg[nodes], kind="stable")]
        l0 = np.zeros(NB, np.int64)
        l1 = np.zeros(NB, np.int64)
        cnt = np.zeros(NB, np.int64)
        for n in nodes:
            cost = np.maximum(l0 + deg0[n], l1 + deg1[n])
            cost[cnt >= P] = 1 << 60
            b = int(np.argmin(cost))
            blk_of[n] = b
            loc_of[n] = cnt[b]
            perm[c, b * P + cnt[b]] = n
            l0[b] += deg0[n]
            l1[b] += deg1[n]
            cnt[b] += 1

    core = core_e
    lb = blk_of[row].astype(np.int64)
    ld = loc_of[row].astype(np.int64)
    half = half_e

    grp = (core * NB + lb) * 2 + half                  # [Et] in [0, C*NB*2)
    order = np.argsort(grp * 128 + ld, kind="stable")  # group, dest-sorted inside
    gs = grp[order]
    counts = np.bincount(grp, minlength=C * NB * 2)
    CHH = max(1, int(math.ceil(counts.max() / P)))     # chunks per half-gather
    SPH = CHH * P                                      # slots per half
    starts = np.zeros(C * NB * 2, np.int64)
    starts[1:] = np.cumsum(counts)[:-1]
    pos = np.arange(Et) - starts[gs]
    slot = gs * SPH + pos

    idx_all = np.full(C * NB * 2 * SPH, HALF, np.int16)   # pad -> zeros row
    idx_all[slot] = (srow - half * HALF)[order].astype(np.int16)
    dest_all = np.full(C * NB * 2 * SPH, -1.0, np.float32)
    dest_all[slot] = ld[order].astype(np.float32)

    idx_all = idx_all.reshape(C, NB * 2, CHH * 8, 16)
    idxg = np.tile(idx_all.transpose(0, 3, 1, 2).reshape(C, 16, NB * 2 * CHH * 8),
                   (1, 8, 1))                              # [C, 128, NB*2*CHH*8]
    destc = dest_all.reshape(C, NB * 2, CHH, P).transpose(0, 3, 1, 2) \
                    .reshape(C, P, NB * 2 * CHH)           # [C, 128, NB*2*CHH]

    xT = np.ascontiguousarray(x.T.astype(NP_F16))          # [128, N] fp16
    xq = np.zeros((C, P, NB * P), NP_F16)
    for c in range(C):
        valid = perm[c] >= 0
        xq[c][:, valid] = xT[:, perm[c][valid]]

    wcat = np.concatenate(
        [np.asarray(query_kernel), np.asarray(key_kernel), np.asarray(kernel)],
        axis=1).astype(NP_F16)                             # [128, 256] fp16
    qkb = np.tile(np.concatenate([np.asarray(query_bias), np.asarray(key_bias)])
                  .astype(np.float32)[None, :], (P, 1))    # [128, 128]
    outb = np.tile(np.asarray(bias, np.float32)[None, :], (P, 1))
    nobias = bool(np.all(qkb == 0.0) and np.all(outb == 0.0))

    in_maps = []
    for c in range(C):
        in_maps.append({
            "xT": np.roll(xT, -int(offs[c]), axis=1),   # table row r = node (r+off_c)%N
            "xq": xq[c], "wcat": wcat, "qkb": qkb, "outb": outb,
            "idxg": np.ascontiguousarray(idxg[c]),
            "destc": np.ascontiguousarray(destc[c]),
        })
    return in_maps, CHH, perm, nobias


# ---------------------------------------------------------------- bass program
def build_program(CHH, reps=1, nobias=False):
    nc = bacc.Bacc(None, target_bir_lowering=False, debug=False,
                   num_swdge_queues=4, dynamic_dma_scratch_size=32768)

    xT = nc.dram_tensor("xT", [P, N], F16, kind="ExternalInput")
    xq = nc.dram_tensor("xq", [P, NB * P], F16, kind="ExternalInput")
    wcat_d = nc.dram_tensor("wcat", [P, 256], F16, kind="ExternalInput")
    qkb_d = nc.dram_tensor("qkb", [P, P], F32, kind="ExternalInput")
    outb_d = nc.dram_tensor("outb", [P, P], F32, kind="ExternalInput")
    idxg_d = nc.dram_tensor("idxg", [P, NB * 2 * CHH * 8], mybir.dt.int16,
                            kind="ExternalInput")
    destc_d = nc.dram_tensor("destc", [P, NB * 2 * CHH], F32, kind="ExternalInput")
    out_d = nc.dram_tensor("out", [NB * P, U], F32, kind="ExternalOutput")

    KV0 = nc.dram_tensor("KV0", [HALF + 1, KVROW], BF16)
    KV1 = nc.dram_tensor("KV1", [N - HALF + 1, KVROW], BF16)

    def store_rows(r0, nr, sb_ap):
        """Store [nr, KVROW] bf16 rows starting at global row r0 into KV0/KV1."""
        if r0 >= HALF:
            nc.sync.dma_start(KV1[r0 - HALF: r0 - HALF + nr], sb_ap)
        elif r0 + nr <= HALF:
            nc.sync.dma_start(KV0[r0: r0 + nr], sb_ap)
        else:
            k = HALF - r0
            nc.sync.dma_start(KV0[r0:HALF], sb_ap[0:k])
            nc.sync.dma_start(KV1[0: nr - k], sb_ap[k:nr])

    with tile.TileContext(nc) as tc:
        with tc.tile_pool(name="const", bufs=1) as cpool:
            # resident tensors
            wcat = cpool.tile([P, 256], F16, tag="wcat")
            qkb = cpool.tile([P, P], F32, tag="qkb")
            outb = cpool.tile([P, P], F32, tag="outb")
            idxg = cpool.tile([P, NB * 2 * CHH * 8], mybir.dt.int16, tag="idxg")
            destc = cpool.tile([P, NB * 2 * CHH], F32, tag="destc")
            qall = cpool.tile([P, NB, A], F16, tag="qall")
            iota_f = cpool.tile([P, P], F32, tag="iotaf")
            iota_b = cpool.tile([P, P], BF16, tag="iotab")
            iota_q = cpool.tile([P, P, GS], BF16, tag="iotaq")
            iota_qi = cpool.tile([P, P, GS], mybir.dt.int32, tag="iotaqi")
            destc_b = cpool.tile([P, NB * 2 * CHH], BF16, tag="destcb")
            ident = cpool.tile([P, P], BF16, tag="ident")
            iota_i = cpool.tile([P, P], mybir.dt.int32, tag="iotai")
            zrow = cpool.tile([1, KVROW], BF16, tag="zrow")

            nc.sync.dma_start(wcat[:], wcat_d[:])
            nc.sync.dma_start(qkb[:], qkb_d[:])
            nc.sync.dma_start(outb[:], outb_d[:])
            nc.sync.dma_start(idxg[:], idxg_d[:])
            nc.sync.dma_start(destc[:], destc_d[:])
            nc.gpsimd.iota(iota_i[:], pattern=[[1, P]], base=0, channel_multiplier=0)
            nc.vector.tensor_copy(iota_f[:], iota_i[:])
            nc.vector.tensor_copy(iota_b[:], iota_i[:])
            nc.gpsimd.iota(iota_qi[:], pattern=[[1, P], [0, GS]], base=0,
                           channel_multiplier=0)
            nc.vector.tensor_copy(iota_q[:], iota_qi[:])
            nc.vector.tensor_copy(destc_b[:], destc[:])
            from concourse.masks import make_identity
            make_identity(nc, ident[:])
            nc.vector.memset(zrow[:], 0.0)
            store_rows(HALF, 1, zrow[:])      # zeros row of KV0 (row HALF==25000)
            nc.sync.dma_start(KV1[N - HALF], zrow[:])  # zeros row of KV1

            for rep in range(reps):
                _emit_pipeline(nc, tc, CHH, xT, xq, out_d, KV0, KV1, store_rows,
                               wcat, qkb, outb, idxg, destc_b, qall, iota_q, ident,
                               rep, nobias)

    nc.compile()
    return nc


def _emit_pipeline(nc, tc, CHH, xT, xq, out_d, KV0, KV1, store_rows,
                   wcat, qkb, outb, idxg, destc, qall, iota_f, ident, rep,
                   nobias=False):
    r = f"r{rep}"
    # ---------------- phase 1a: Q for local nodes -> qall (f32)
    with (
        tc.tile_pool(name=f"qx{r}", bufs=3) as qxp,
        tc.tile_pool(name=f"qps{r}", bufs=2, space="PSUM") as qpsp,
        tc.tile_pool(name=f"qtmp{r}", bufs=3) as qtp,
    ):
        for b in range(NB):
            xqt = qxp.tile([P, P], F16, tag="xqt")
            nc.sync.dma_start(xqt[:], xq[:, b * P:(b + 1) * P])
            qps = qpsp.tile([P, A], F32, tag="qps")
            nc.tensor.matmul(qps[:], xqt[:], wcat[:, 0:A],
                             start=True, stop=True)
            if nobias:
                nc.scalar.activation(qall[:, b, :], qps[:],
                                     mybir.ActivationFunctionType.Relu)
            else:
                qtmp = qtp.tile([P, A], F32, tag="qtmp")
                nc.vector.tensor_tensor(out=qtmp[:], in0=qps[:],
                                        in1=qkb[:, 0:A],
                                        op=mybir.AluOpType.add)
                nc.scalar.activation(qall[:, b, :], qtmp[:],
                                     mybir.ActivationFunctionType.Relu)

    # ---------------- interleaved phase 1b (KV table build) + phase 2.
    # Phase 2 runs two passes over source halves: pass 0 needs only KV0
    # (built first), pass 1 needs KV1. Emission order interleaves the KV1
    # tile builds with pass-0 blocks so every engine stream makes progress;
    # pool allocation keeps phase-2 tiles disjoint from phase-1b tiles so
    # no address-reuse WAR serializes the overlap. Gathers round-robin the
    # 4 SWDGE queues (disjoint Q7 core pairs -> concurrent descriptor gen).
    XW = 512
    NT = math.ceil(N / XW)
    NT0 = math.ceil(HALF / XW)
    groups = []
    c = 0
    while c < CHH:
        gn = min(GS, CHH - c)
        groups.append((c, gn))
        c += gn
    rrq = [0]
    with (
        tc.tile_pool(name=f"kvt{r}", bufs=6) as kvtp,
        tc.tile_pool(name=f"oh{r}", bufs=4) as ohp,
        tc.tile_pool(name=f"ohtps{r}", bufs=2, space="PSUM") as ohtpsp,
        tc.tile_pool(name=f"oht{r}", bufs=4) as ohtp,
        tc.tile_pool(name=f"qeps{r}", bufs=2, space="PSUM") as qepsp,
        tc.tile_pool(name=f"prod{r}", bufs=4) as prp,
        tc.tile_pool(name=f"score{r}", bufs=4) as scp,
        tc.tile_pool(name=f"wt{r}", bufs=4) as wtp,
        tc.tile_pool(name=f"ops{r}", bufs=2, space="PSUM") as opsp,
        tc.tile_pool(name=f"part{r}", bufs=1) as partp,
        tc.tile_pool(name=f"fin{r}", bufs=3) as finp,
        tc.tile_pool(name=f"xload{r}", bufs=3) as xlp,
        tc.tile_pool(name=f"kvps{r}", bufs=2, space="PSUM") as kvpsp,
        tc.tile_pool(name=f"kvsb{r}", bufs=3) as kvsbp,
        tc.tile_pool(name=f"ktmp{r}", bufs=2) as ktp,
    ):
        parts = []

        def emit_kv_tile(t):
            n0 = t * XW
            nn = min(XW, N - n0)
            ns = math.ceil(nn / P)          # subtiles (4, last tile 3)
            xt = xlp.tile([P, XW], F16, tag="xt")
            nc.sync.dma_start(xt[:, 0:nn], xT[:, n0:n0 + nn])
            kvsb = kvsbp.tile([P, 4, KVROW], BF16, tag="kvsb")
            ktmp = ktp.tile([P, 4, A], F32, tag="ktmp")
            pss = []
            for pair in range(math.ceil(ns / 2)):
                ps = kvpsp.tile([P, 2, 192], F32, tag="kvps")
                pss.append(ps)
                for j in range(min(2, ns - 2 * pair)):
                    s = 2 * pair + j
                    nr = min(P, nn - s * P)
                    nc.tensor.matmul(ps[0:nr, j, :], xt[:, s * P: s * P + nr],
                                     wcat[:, A:256], start=True, stop=True)
            for pair in range(math.ceil(ns / 2)):
                np_ = min(2, ns - 2 * pair)
                ps = pss[pair]
                sl = slice(2 * pair, 2 * pair + np_)
                if nobias:
                    nc.vector.tensor_scalar_max(
                        kvsb[:, sl, 0:U].bitcast(F32), ps[:, 0:np_, 0:A], 0.0)
                else:
                    nc.vector.tensor_tensor(
                        out=ktmp[:, sl, :], in0=ps[:, 0:np_, 0:A],
                        in1=qkb[:, None, A:P].broadcast_to([P, np_, A]),
                        op=mybir.AluOpType.add)
                nc.scalar.copy(kvsb[:, sl, U:KVROW], ps[:, 0:np_, A:192])
            if not nobias:
                nc.scalar.activation(kvsb[:, 0:ns, 0:U].bitcast(F32),
                                     ktmp[:, 0:ns, :],
                                     mybir.ActivationFunctionType.Relu)
            # batched store of [P, ns, KVROW] via ACT dispatch (sync is the
            # phase-1 bottleneck otherwise): rows n0 + s*128 + p
            full = nn == ns * P
            lo, hi = n0, n0 + nn
            if hi <= HALF or lo >= HALF:
                dst, off = (KV0, 0) if hi <= HALF else (KV1, HALF)
                if full:
                    nc.scalar.dma_start(
                        dst[lo - off: hi - off].rearrange("(s p) e -> p s e", p=P),
                        kvsb[:, 0:ns, :])
                else:
                    nfull = nn // P
                    if nfull:
                        nc.scalar.dma_start(
                            dst[lo - off: lo - off + nfull * P]
                            .rearrange("(s p) e -> p s e", p=P),
                            kvsb[:, 0:nfull, :])
                    rem = nn - nfull * P
                    nc.scalar.dma_start(
                        dst[lo - off + nfull * P: lo - off + nn],
                        kvsb[0:rem, nfull, :])
            else:
                # crosses the HALF split: store per subtile
                for s in range(ns):
                    r0 = n0 + s * P
                    nr = min(P, N - r0)
                    store_rows(r0, nr, kvsb[0:nr, s, :])

        def emit_block(b, hf):
            srct = KV0 if hf == 0 else KV1
            kvt = kvtp.tile([P, CHH, KVROW], BF16, tag="kvt")
            i0 = (b * 2 + hf) * CHH * 8
            nc.gpsimd.dma_gather(
                kvt[:], srct[:], idxg[:, i0: i0 + CHH * 8],
                num_idxs=CHH * P, num_idxs_reg=CHH * P,
                elem_size=KVROW, single_packet=False,
                queue_num=rrq[0] % 4,
            )
            rrq[0] += 1
            ops = opsp.tile([P, U + H], F32, tag="ops")
            for (c0, gn) in groups:
                g0 = (b * 2 + hf) * CHH + c0
                oh = ohp.tile([P, P, GS], BF16, tag="oh")
                nc.vector.tensor_tensor(
                    out=oh[:, :, 0:gn],
                    in0=destc[:, g0:g0 + gn][:, None, :]
                        .broadcast_to([P, P, gn]),
                    in1=iota_f[:, :, 0:gn],
                    op=mybir.AluOpType.is_equal)
                ohtps = ohtpsp.tile([P, GS, P], BF16, tag="ohtps")
                for j in range(gn):
                    nc.tensor.transpose(ohtps[:, j, :], oh[:, :, j], ident[:])
                oht = ohtp.tile([P, GS, P], F16, tag="oht")
                nc.scalar.copy(oht[:, 0:gn, :], ohtps[:, 0:gn, :])
                qeps = qepsp.tile([P, GS, A], F32, tag="qeps")
                for j in range(gn):
                    nc.tensor.matmul(qeps[:, j, :], oht[:, j, :],
                                     qall[:, b, :], start=True, stop=True)
                prod = prp.tile([P, GS, A], F32, tag="prod")
                nc.vector.tensor_tensor(
                    out=prod[:, 0:gn, :], in0=qeps[:, 0:gn, :],
                    in1=kvt[:, c0:c0 + gn, 0:U].bitcast(F32),
                    op=mybir.AluOpType.mult)
                score = scp.tile([P, GS, H], F32, tag="score")
                nc.vector.tensor_reduce(
                    score[:, 0:gn, :],
                    prod[:, 0:gn, :].rearrange("p q (h d) -> p q h d", h=H),
                    axis=mybir.AxisListType.X, op=mybir.AluOpType.add)
                wt = wtp.tile([P, GS, U + H], BF16, tag="wt")
                nc.scalar.activation(wt[:, 0:gn, U:U + H],
                                     score[:, 0:gn, :],
                                     mybir.ActivationFunctionType.Exp)
                nc.vector.tensor_tensor(
                    out=wt[:, 0:gn, 0:U].rearrange("p q (h u) -> p q h u",
                                                   h=H),
                    in0=kvt[:, c0:c0 + gn, U:KVROW]
                        .rearrange("p q (h u) -> p q h u", h=H),
                    in1=wt[:, 0:gn, U:U + H][:, :, :, None]
                        .broadcast_to([P, gn, H, UD]),
                    op=mybir.AluOpType.mult)
                for j in range(gn):
                    ch = c0 + j
                    nc.tensor.matmul(ops[:], oh[:, :, j], wt[:, j, :],
                                     start=(ch == 0), stop=(ch == CHH - 1))
            if hf == 0:
                part = partp.tile([P, U + H], F32, tag=f"part{b}")
                parts.append(part)
                nc.scalar.copy(part[:], ops[:])
            else:
                tot = finp.tile([P, U + H], F32, tag="tot")
                nc.vector.tensor_tensor(out=tot[:], in0=ops[:],
                                        in1=parts[b][:],
                                        op=mybir.AluOpType.add)
                recip = finp.tile([P, H], F32, tag="recip")
                nc.vector.reciprocal(recip[:], tot[:, U:U + H])
                o1 = finp.tile([P, U], F32, tag="o1")
                nc.vector.tensor_tensor(
                    out=o1[:].rearrange("p (h u) -> p h u", h=H),
                    in0=tot[:, 0:U].rearrange("p (h u) -> p h u", h=H),
                    in1=recip[:][:, :, None].broadcast_to([P, H, UD]),
                    op=mybir.AluOpType.mult)
                if not nobias:
                    nc.vector.tensor_tensor(out=o1[:], in0=o1[:],
                                            in1=outb[:],
                                            op=mybir.AluOpType.add)
                nc.sync.dma_start(out_d[b * P: (b + 1) * P], o1[:])

        for t in range(NT0):
            emit_kv_tile(t)
        for b in range(NB):
            emit_block(b, 0)
            t = NT0 + b
            if t < NT:
                emit_kv_tile(t)
        for b in range(NB):
            emit_block(b, 1)


# ---------------------------------------------------------------- execution
class SpmdRunner:
    def __init__(self, nc, n_cores=C):
        import jax
        from jax.sharding import Mesh, PartitionSpec
        from jax.experimental.shard_map import shard_map
        from concourse.bass2jax import (_bass_exec_p, install_neuronx_cc_hook,
                                        partition_id_tensor)
        install_neuronx_cc_hook()
        self.jax = jax
        self.nc = nc
        self.n_cores = n_cores
        partition_name = nc.partition_id_tensor.name if nc.partition_id_tensor else None
        in_names, out_names, out_avals = [], [], []
        for alloc in nc.m.functions[0].allocations:
            if not isinstance(alloc, mybir.MemoryLocationSet):
                continue
            name = alloc.memorylocations[0].name
            if alloc.kind == "ExternalInput":
                if name != partition_name:
                    in_names.append(name)
            elif alloc.kind == "ExternalOutput":
                out_names.append(name)
                out_avals.append(jax.core.ShapedArray(
                    tuple(alloc.tensor_shape), mybir.dt.np(alloc.dtype)))
        self.in_names, self.out_names, self.out_avals = in_names, out_names, out_avals
        n_params = len(in_names)

        all_in_names = list(in_names) + list(out_names)
        if partition_name is not None:
            all_in_names.append(partition_name)

        def _body(*args):
            operands = list(args)
            if partition_name is not None:
                operands.append(partition_id_tensor())
            outs = _bass_exec_p.bind(
                *operands,
                out_avals=tuple(out_avals),
                in_names=tuple(all_in_names),
                out_names=tuple(out_names),
                lowering_input_output_aliases=(),
                sim_require_finite=False,
                sim_require_nnan=False,
                nc=nc,
            )
            return tuple(outs)

        devices = jax.devices()[:n_cores]
        self.mesh = Mesh(np.asarray(devices), ("core",))
        n_extra = len(out_names)
        in_specs = (PartitionSpec("core"),) * (n_params + n_extra)
        out_specs = (PartitionSpec("core"),) * len(out_names)
        self.fn = jax.jit(
            shard_map(_body, mesh=self.mesh, in_specs=in_specs,
                      out_specs=out_specs, check_rep=False),
            keep_unused=True,
        )

    def put_inputs(self, in_maps):
        from jax.sharding import NamedSharding, PartitionSpec
        sharding = NamedSharding(self.mesh, PartitionSpec("core"))
        args = []
        for name in self.in_names:
            concat = np.concatenate([np.asarray(m[name]) for m in in_maps], axis=0)
            args.append(self.jax.device_put(concat, sharding))
        for av in self.out_avals:
            args.append(self.jax.device_put(
                np.zeros((self.n_cores * av.shape[0], *av.shape[1:]), av.dtype),
                sharding))
        return args

    def __call__(self, args):
        outs = self.fn(*args)
        self.jax.block_until_ready(outs)
        return outs

    def run_to_numpy(self, args):
        outs = self(args)
        res = []
        for c in range(self.n_cores):
            d = {}
            for i, name in enumerate(self.out_names):
                d[name] = np.asarray(outs[i]).reshape(
                    self.n_cores, *self.out_avals[i].shape)[c]
            res.append(d)
        return res


_CACHE = {}


def _get_runner(CHH, reps=1, nobias=False):
    key = (CHH, reps, nobias)
    if key not in _CACHE:
        nc = build_program(CHH, reps=reps, nobias=nobias)
        _CACHE[key] = SpmdRunner(nc)
    return _CACHE[key]


def kernel(x, edge_index, query_kernel, query_bias, key_kernel, key_bias,
           kernel, bias):
    in_maps, CHH, perm, nobias = preprocess(x, edge_index, query_kernel,
                                            query_bias, key_kernel, key_bias,
                                            kernel, bias)
    runner = _get_runner(CHH, nobias=nobias)
    args = runner.put_inputs(in_maps)
    res = runner.run_to_numpy(args)
    out = np.empty((N, U), np.float32)
    for c in range(C):
        valid = perm[c] >= 0
        out[perm[c][valid]] = res[c]["out"][valid]
    return out



# revision 18
# speedup vs baseline: 2.6137x; 1.3832x over previous
"""GAT message-passing kernel for Trainium2, 8 NeuronCores (graph-parallel).

Contract: kernel(**inputs) takes FULL inputs (x [50000,128] f32,
edge_index [2,800000] i32, weights/biases) and returns the FULL output
[50000, 128] f32. Self-contained: preprocessing (numpy) + Bass program +
PJRT exec are all in this file.

Sharding / algorithm (per core, destinations sharded 6250/core):
- Host: add self-loops; LPT-pack each core's destinations into 49 blocks of
  <=128 so per-(block, half) edge counts are balanced; bucket+sort edges by
  (block, source-half); emit int16 gather indices (wrapped [16 x n/16],
  replicated across the 8 Q7 cores) and per-chunk block-local dest ids.
- Phase 1 (dense, redundant on every core): K=relu(x@Wk+kb), V=x@W from a
  host-pretransposed fp16 xT via one 192-col matmul per 128-node tile,
  packed into two half-tables (25001 rows each, int16-indexable, + a zeros
  row for padding) of 512B rows [K as f32 | V as bf16]; Q=relu(x@Wq+qb) for
  local nodes only, SBUF-resident.
- Phase 2 (attention, per 128-dest block): dma_gather the block's edge
  sources (2 gathers, one per half-table); per 128-edge chunk build the
  one-hot OH[e,d] with a DVE is_equal against an iota (chunk-minor layout to
  hit the 2x DVE mode; pad edges carry dest=-1 so their one-hot rows are
  zero -> self-masking), PE-transpose it, expand Q to edges with one matmul,
  score = per-head reduce of Q*K (K read back as f32), exp on ACT (bf16),
  scale V by exp, then a single PSUM-accumulated matmul per chunk computes
  both sum(exp*V) and sum(exp) (concatenated rhs). Normalize + bias at block
  end; host inverse-permutes the balanced block layout.
Softmax max-subtraction is dropped (scores ~O(30) max, exp stays in fp32
range; matches the reference exactly up to rounding).
"""
import math
import os

import numpy as np

import ml_dtypes

import concourse.bass as bass
import concourse.mybir as mybir
import concourse.tile as tile
from concourse import bacc

P = 128
C = 8                    # cores
N, F, E = 50000, 128, 800000
H, A, U = 8, 64, 128     # heads, att units, units
HD = A // H              # per-head q/k dim (8)
UD = U // H              # per-head v dim (16)
NPC = N // C             # nodes per core
NB = math.ceil(NPC / P)  # dest blocks per core (49)
HALF = N // 2            # table split point (fits int16 indices)
BF16 = mybir.dt.bfloat16
F32 = mybir.dt.float32
NP_BF16 = ml_dtypes.bfloat16
F16 = mybir.dt.float16
NP_F16 = np.float16

KVROW = 256              # bf16 elems per packed row: [K as f32-bitcast (128) | V bf16 (128)] = 512B
GS = 5                   # chunks per attention instruction group


# ---------------------------------------------------------------- preprocessing
def preprocess(x, edge_index, query_kernel, query_bias, key_kernel, key_bias,
               kernel, bias):
    """Build per-core input maps + the uniform structure params.

    Destinations are assigned to (core, block) with an LPT greedy pack so
    per-block edge counts are balanced -> minimal chunk padding. Returns
    (in_maps, CHH, perm) where perm[c, b*P+i] is the global node id stored
    at output row (c, b*P+i), or -1 for unused slots.
    """
    import heapq
    x = np.asarray(x, np.float32)
    ei = np.asarray(edge_index, np.int64)
    row = np.concatenate([ei[0], np.arange(N, dtype=np.int64)])   # dest
    col = np.concatenate([ei[1], np.arange(N, dtype=np.int64)])   # src
    Et = row.shape[0]

    # per-core source relabeling: core c stores node n's KV row at
    # (n - off_c) mod N with off_c = c*NPC - (HALF - NPC//2), so each core's
    # self-loop sources straddle the KV0/KV1 split -> halves stay balanced.
    offs = np.array([c * NPC - (HALF - NPC // 2) for c in range(C)])
    core_e = row // NPC
    srow = (col - offs[core_e]) % N
    half_e = (srow >= HALF).astype(np.int64)
    deg0 = np.bincount(row[half_e == 0], minlength=N)
    deg1 = np.bincount(row[half_e == 1], minlength=N)
    deg = deg0 + deg1
    # --- balanced block assignment per core: greedy pack minimizing the max
    # per-(block, half) load (that max sets CHH = the gather chunk count) ---
    blk_of = np.empty(N, np.int32)
    loc_of = np.empty(N, np.int32)
    perm = np.full((C, NB * P), -1, np.int64)
    for c in range(C):
        nodes = np.arange(c * NPC, (c + 1) * NPC)
        nodes = nodes[np.argsort(-deg[nodes], kind="stable")]
        l0 = np.zeros(NB, np.int64)
        l1 = np.zeros(NB, np.int64)
        cnt = np.zeros(NB, np.int64)
        for n in nodes:
            cost = np.maximum(l0 + deg0[n], l1 + deg1[n])
            cost[cnt >= P] = 1 << 60
            b = int(np.argmin(cost))
            blk_of[n] = b
            loc_of[n] = cnt[b]
            perm[c, b * P + cnt[b]] = n
            l0[b] += deg0[n]
            l1[b] += deg1[n]
            cnt[b] += 1

    core = core_e
    lb = blk_of[row].astype(np.int64)
    ld = loc_of[row].astype(np.int64)
    half = half_e

    grp = (core * NB + lb) * 2 + half                  # [Et] in [0, C*NB*2)
    order = np.argsort(grp * 128 + ld, kind="stable")  # group, dest-sorted inside
    gs = grp[order]
    counts = np.bincount(grp, minlength=C * NB * 2)
    CHH = max(1, int(math.ceil(counts.max() / P)))     # chunks per half-gather
    SPH = CHH * P                                      # slots per half
    starts = np.zeros(C * NB * 2, np.int64)
    starts[1:] = np.cumsum(counts)[:-1]
    pos = np.arange(Et) - starts[gs]
    slot = gs * SPH + pos

    idx_all = np.full(C * NB * 2 * SPH, HALF, np.int16)   # pad -> zeros row
    idx_all[slot] = (srow - half * HALF)[order].astype(np.int16)
    dest_all = np.full(C * NB * 2 * SPH, -1.0, np.float32)
    dest_all[slot] = ld[order].astype(np.float32)

    idx_all = idx_all.reshape(C, NB * 2, CHH * 8, 16)
    idxg = np.tile(idx_all.transpose(0, 3, 1, 2).reshape(C, 16, NB * 2 * CHH * 8),
                   (1, 8, 1))                              # [C, 128, NB*2*CHH*8]
    destc = dest_all.reshape(C, NB * 2, CHH, P).transpose(0, 3, 1, 2) \
                    .reshape(C, P, NB * 2 * CHH)           # [C, 128, NB*2*CHH]

    xT = np.ascontiguousarray(x.T.astype(NP_F16))          # [128, N] fp16
    xq = np.zeros((C, P, NB * P), NP_F16)
    for c in range(C):
        valid = perm[c] >= 0
        xq[c][:, valid] = xT[:, perm[c][valid]]

    # V columns permuted head-minor (u' = ud*H + h) so the on-device exp
    # broadcast in the wt multiply is innermost-contiguous (2x DVE packing);
    # kernel() un-permutes the output columns.
    uperm = np.array([h * UD + ud for ud in range(UD) for h in range(H)])
    wcat = np.concatenate(
        [np.asarray(query_kernel), np.asarray(key_kernel),
         np.asarray(kernel)[:, uperm]],
        axis=1).astype(NP_F16)                             # [128, 256] fp16
    qkb = np.tile(np.concatenate([np.asarray(query_bias), np.asarray(key_bias)])
                  .astype(np.float32)[None, :], (P, 1))    # [128, 128]
    outb = np.tile(np.asarray(bias, np.float32)[None, uperm], (P, 1))
    nobias = bool(np.all(qkb == 0.0) and np.all(outb == 0.0))

    in_maps = []
    for c in range(C):
        in_maps.append({
            "xT": np.roll(xT, -int(offs[c]), axis=1),   # table row r = node (r+off_c)%N
            "xq": xq[c], "wcat": wcat, "qkb": qkb, "outb": outb,
            "idxg": np.ascontiguousarray(idxg[c]),
            "destc": np.ascontiguousarray(destc[c]),
        })
    return in_maps, CHH, perm, nobias


# ---------------------------------------------------------------- bass program
def build_program(CHH, reps=1, nobias=False):
    nc = bacc.Bacc(None, target_bir_lowering=False, debug=False,
                   num_swdge_queues=4, dynamic_dma_scratch_size=32768)

    xT = nc.dram_tensor("xT", [P, N], F16, kind="ExternalInput")
    xq = nc.dram_tensor("xq", [P, NB * P], F16, kind="ExternalInput")
    wcat_d = nc.dram_tensor("wcat", [P, 256], F16, kind="ExternalInput")
    qkb_d = nc.dram_tensor("qkb", [P, P], F32, kind="ExternalInput")
    outb_d = nc.dram_tensor("outb", [P, P], F32, kind="ExternalInput")
    idxg_d = nc.dram_tensor("idxg", [P, NB * 2 * CHH * 8], mybir.dt.int16,
                            kind="ExternalInput")
    destc_d = nc.dram_tensor("destc", [P, NB * 2 * CHH], F32, kind="ExternalInput")
    out_d = nc.dram_tensor("out", [NB * P, U], F32, kind="ExternalOutput")

    KV0 = nc.dram_tensor("KV0", [HALF + 1, KVROW], BF16)
    KV1 = nc.dram_tensor("KV1", [N - HALF + 1, KVROW], BF16)

    def store_rows(r0, nr, sb_ap):
        """Store [nr, KVROW] bf16 rows starting at global row r0 into KV0/KV1."""
        if r0 >= HALF:
            nc.sync.dma_start(KV1[r0 - HALF: r0 - HALF + nr], sb_ap)
        elif r0 + nr <= HALF:
            nc.sync.dma_start(KV0[r0: r0 + nr], sb_ap)
        else:
            k = HALF - r0
            nc.sync.dma_start(KV0[r0:HALF], sb_ap[0:k])
            nc.sync.dma_start(KV1[0: nr - k], sb_ap[k:nr])

    with tile.TileContext(nc) as tc:
        with tc.tile_pool(name="const", bufs=1) as cpool:
            # resident tensors
            wcat = cpool.tile([P, 256], F16, tag="wcat")
            qkb = cpool.tile([P, P], F32, tag="qkb")
            outb = cpool.tile([P, P], F32, tag="outb")
            idxg = cpool.tile([P, NB * 2 * CHH * 8], mybir.dt.int16, tag="idxg")
            destc = cpool.tile([P, NB * 2 * CHH], F32, tag="destc")
            qall = cpool.tile([P, NB, A], F16, tag="qall")
            iota_f = cpool.tile([P, P], F32, tag="iotaf")
            iota_b = cpool.tile([P, P], BF16, tag="iotab")
            iota_q = cpool.tile([P, P, CHH], BF16, tag="iotaq")
            iota_qi = cpool.tile([P, P, CHH], mybir.dt.int32, tag="iotaqi")
            destc_b = cpool.tile([P, NB * 2 * CHH], BF16, tag="destcb")
            ident = cpool.tile([P, P], BF16, tag="ident")
            iota_i = cpool.tile([P, P], mybir.dt.int32, tag="iotai")
            zrow = cpool.tile([1, KVROW], BF16, tag="zrow")

            nc.sync.dma_start(wcat[:], wcat_d[:])
            nc.sync.dma_start(qkb[:], qkb_d[:])
            nc.sync.dma_start(outb[:], outb_d[:])
            nc.sync.dma_start(idxg[:], idxg_d[:])
            nc.sync.dma_start(destc[:], destc_d[:])
            nc.gpsimd.iota(iota_i[:], pattern=[[1, P]], base=0, channel_multiplier=0)
            nc.vector.tensor_copy(iota_f[:], iota_i[:])
            nc.vector.tensor_copy(iota_b[:], iota_i[:])
            nc.gpsimd.iota(iota_qi[:], pattern=[[1, P], [0, CHH]], base=0,
                           channel_multiplier=0)
            nc.vector.tensor_copy(iota_q[:], iota_qi[:])
            nc.vector.tensor_copy(destc_b[:], destc[:])
            from concourse.masks import make_identity
            make_identity(nc, ident[:])
            nc.vector.memset(zrow[:], 0.0)
            store_rows(HALF, 1, zrow[:])      # zeros row of KV0 (row HALF==25000)
            nc.sync.dma_start(KV1[N - HALF], zrow[:])  # zeros row of KV1

            for rep in range(reps):
                _emit_pipeline(nc, tc, CHH, xT, xq, out_d, KV0, KV1, store_rows,
                               wcat, qkb, outb, idxg, destc_b, qall, iota_q, ident,
                               rep, nobias)

    nc.compile()
    return nc


def _emit_pipeline(nc, tc, CHH, xT, xq, out_d, KV0, KV1, store_rows,
                   wcat, qkb, outb, idxg, destc, qall, iota_f, ident, rep,
                   nobias=False):
    r = f"r{rep}"
    # ---------------- phase 1a: Q for local nodes -> qall (f32)
    with (
        tc.tile_pool(name=f"qx{r}", bufs=3) as qxp,
        tc.tile_pool(name=f"qps{r}", bufs=2, space="PSUM") as qpsp,
        tc.tile_pool(name=f"qtmp{r}", bufs=3) as qtp,
    ):
        for b in range(NB):
            xqt = qxp.tile([P, P], F16, tag="xqt")
            nc.sync.dma_start(xqt[:], xq[:, b * P:(b + 1) * P])
            qps = qpsp.tile([P, A], F32, tag="qps")
            nc.tensor.matmul(qps[:], xqt[:], wcat[:, 0:A],
                             start=True, stop=True)
            if nobias:
                nc.scalar.activation(qall[:, b, :], qps[:],
                                     mybir.ActivationFunctionType.Relu)
            else:
                qtmp = qtp.tile([P, A], F32, tag="qtmp")
                nc.vector.tensor_tensor(out=qtmp[:], in0=qps[:],
                                        in1=qkb[:, 0:A],
                                        op=mybir.AluOpType.add)
                nc.scalar.activation(qall[:, b, :], qtmp[:],
                                     mybir.ActivationFunctionType.Relu)

    # ---------------- interleaved phase 1b (KV table build) + phase 2.
    # Phase 2 runs two passes over source halves: pass 0 needs only KV0
    # (built first), pass 1 needs KV1. Emission order interleaves the KV1
    # tile builds with pass-0 blocks so every engine stream makes progress;
    # pool allocation keeps phase-2 tiles disjoint from phase-1b tiles so
    # no address-reuse WAR serializes the overlap. Gathers round-robin the
    # 4 SWDGE queues (disjoint Q7 core pairs -> concurrent descriptor gen).
    XW = 512
    NT = math.ceil(N / XW)
    NT0 = math.ceil(HALF / XW)
    groups = []
    c = 0
    while c < CHH:
        gn = min(GS, CHH - c)
        groups.append((c, gn))
        c += gn
    rrq = [0]
    with (
        tc.tile_pool(name=f"kvt{r}", bufs=8) as kvtp,
        tc.tile_pool(name=f"oh{r}", bufs=4) as ohp,
        tc.tile_pool(name=f"ohtps{r}", bufs=2, space="PSUM") as ohtpsp,
        tc.tile_pool(name=f"oht{r}", bufs=4) as ohtp,
        tc.tile_pool(name=f"qeps{r}", bufs=2, space="PSUM") as qepsp,
        tc.tile_pool(name=f"prod{r}", bufs=4) as prp,
        tc.tile_pool(name=f"score{r}", bufs=4) as scp,
        tc.tile_pool(name=f"wt{r}", bufs=4) as wtp,
        tc.tile_pool(name=f"ops{r}", bufs=2, space="PSUM") as opsp,
        tc.tile_pool(name=f"part{r}", bufs=1) as partp,
        tc.tile_pool(name=f"fin{r}", bufs=3) as finp,
        tc.tile_pool(name=f"xload{r}", bufs=3) as xlp,
        tc.tile_pool(name=f"kvps{r}", bufs=2, space="PSUM") as kvpsp,
        tc.tile_pool(name=f"kvsb{r}", bufs=3) as kvsbp,
        tc.tile_pool(name=f"ktmp{r}", bufs=2) as ktp,
    ):
        parts = []

        def emit_kv_tile(t):
            n0 = t * XW
            nn = min(XW, N - n0)
            ns = math.ceil(nn / P)          # subtiles (4, last tile 3)
            xt = xlp.tile([P, XW], F16, tag="xt")
            nc.sync.dma_start(xt[:, 0:nn], xT[:, n0:n0 + nn])
            kvsb = kvsbp.tile([P, 4, KVROW], BF16, tag="kvsb")
            ktmp = ktp.tile([P, 4, A], F32, tag="ktmp")
            pss = []
            for pair in range(math.ceil(ns / 2)):
                ps = kvpsp.tile([P, 2, 192], F32, tag="kvps")
                pss.append(ps)
                for j in range(min(2, ns - 2 * pair)):
                    s = 2 * pair + j
                    nr = min(P, nn - s * P)
                    nc.tensor.matmul(ps[0:nr, j, :], xt[:, s * P: s * P + nr],
                                     wcat[:, A:256], start=True, stop=True)
            for pair in range(math.ceil(ns / 2)):
                np_ = min(2, ns - 2 * pair)
                ps = pss[pair]
                sl = slice(2 * pair, 2 * pair + np_)
                if nobias:
                    nc.vector.tensor_scalar_max(
                        kvsb[:, sl, 0:U].bitcast(F32), ps[:, 0:np_, 0:A], 0.0)
                else:
                    nc.vector.tensor_tensor(
                        out=ktmp[:, sl, :], in0=ps[:, 0:np_, 0:A],
                        in1=qkb[:, None, A:P].broadcast_to([P, np_, A]),
                        op=mybir.AluOpType.add)
                nc.scalar.copy(kvsb[:, sl, U:KVROW], ps[:, 0:np_, A:192])
            if not nobias:
                nc.scalar.activation(kvsb[:, 0:ns, 0:U].bitcast(F32),
                                     ktmp[:, 0:ns, :],
                                     mybir.ActivationFunctionType.Relu)
            # batched store of [P, ns, KVROW] via ACT dispatch (sync is the
            # phase-1 bottleneck otherwise): rows n0 + s*128 + p
            full = nn == ns * P
            lo, hi = n0, n0 + nn
            if hi <= HALF or lo >= HALF:
                dst, off = (KV0, 0) if hi <= HALF else (KV1, HALF)
                if full:
                    nc.scalar.dma_start(
                        dst[lo - off: hi - off].rearrange("(s p) e -> p s e", p=P),
                        kvsb[:, 0:ns, :])
                else:
                    nfull = nn // P
                    if nfull:
                        nc.scalar.dma_start(
                            dst[lo - off: lo - off + nfull * P]
                            .rearrange("(s p) e -> p s e", p=P),
                            kvsb[:, 0:nfull, :])
                    rem = nn - nfull * P
                    nc.scalar.dma_start(
                        dst[lo - off + nfull * P: lo - off + nn],
                        kvsb[0:rem, nfull, :])
            else:
                # crosses the HALF split: store per subtile
                for s in range(ns):
                    r0 = n0 + s * P
                    nr = min(P, N - r0)
                    store_rows(r0, nr, kvsb[0:nr, s, :])

        def emit_block(b, hf):
            srct = KV0 if hf == 0 else KV1
            kvt = kvtp.tile([P, CHH, KVROW], BF16, tag="kvt")
            i0 = (b * 2 + hf) * CHH * 8
            nc.gpsimd.dma_gather(
                kvt[:], srct[:], idxg[:, i0: i0 + CHH * 8],
                num_idxs=CHH * P, num_idxs_reg=CHH * P,
                elem_size=KVROW, single_packet=False,
                queue_num=rrq[0] % 4,
            )
            rrq[0] += 1
            ops = opsp.tile([P, U + H], F32, tag="ops")
            g0b = (b * 2 + hf) * CHH
            ohall = ohp.tile([P, P, CHH], BF16, tag="ohall")
            nc.vector.tensor_tensor(
                out=ohall[:],
                in0=destc[:, g0b:g0b + CHH][:, None, :]
                    .broadcast_to([P, P, CHH]),
                in1=iota_f[:, :, 0:CHH],
                op=mybir.AluOpType.is_equal)
            for (c0, gn) in groups:
                oh = ohall[:, :, c0:c0 + gn]
                ohtps = ohtpsp.tile([P, GS, P], BF16, tag="ohtps")
                for j in range(gn):
                    nc.tensor.transpose(ohtps[:, j, :], oh[:, :, j], ident[:])
                oht = ohtp.tile([P, GS, P], BF16, tag="oht")
                nc.scalar.copy(oht[:, 0:gn, :].bitcast(F32),
                               ohtps[:, 0:gn, :].bitcast(F32))
                qeps = qepsp.tile([P, GS, A], F32, tag="qeps")
                for j in range(gn):
                    nc.tensor.matmul(qeps[:, j, :], oht[:, j, :],
                                     qall[:, b, :], start=True, stop=True)
                prod = prp.tile([P, GS, A], F32, tag="prod")
                nc.vector.tensor_tensor(
                    out=prod[:, 0:gn, :], in0=qeps[:, 0:gn, :],
                    in1=kvt[:, c0:c0 + gn, 0:U].bitcast(F32),
                    op=mybir.AluOpType.mult)
                score = scp.tile([P, GS, H], F32, tag="score")
                nc.vector.tensor_reduce(
                    score[:, 0:gn, :],
                    prod[:, 0:gn, :].rearrange("p q (h d) -> p q h d", h=H),
                    axis=mybir.AxisListType.X, op=mybir.AluOpType.add)
                wt = wtp.tile([P, GS, U + H], BF16, tag="wt")
                nc.scalar.activation(wt[:, 0:gn, U:U + H],
                                     score[:, 0:gn, :],
                                     mybir.ActivationFunctionType.Exp)
                nc.vector.tensor_tensor(
                    out=wt[:, 0:gn, 0:U].rearrange("p q (u h) -> p q u h",
                                                   h=H),
                    in0=kvt[:, c0:c0 + gn, U:KVROW]
                        .rearrange("p q (u h) -> p q u h", h=H),
                    in1=wt[:, 0:gn, U:U + H][:, :, None, :]
                        .broadcast_to([P, gn, UD, H]),
                    op=mybir.AluOpType.mult)
                for j in range(gn):
                    ch = c0 + j
                    nc.tensor.matmul(ops[:], oh[:, :, j], wt[:, j, :],
                                     start=(ch == 0), stop=(ch == CHH - 1))
            if hf == 0:
                part = partp.tile([P, U + H], F32, tag=f"part{b}")
                parts.append(part)
                nc.scalar.copy(part[:], ops[:])
            else:
                tot = finp.tile([P, U + H], F32, tag="tot")
                nc.vector.tensor_tensor(out=tot[:], in0=ops[:],
                                        in1=parts[b][:],
                                        op=mybir.AluOpType.add)
                recip = finp.tile([P, H], F32, tag="recip")
                nc.vector.reciprocal(recip[:], tot[:, U:U + H])
                o1 = finp.tile([P, U], F32, tag="o1")
                nc.vector.tensor_tensor(
                    out=o1[:].rearrange("p (u h) -> p u h", h=H),
                    in0=tot[:, 0:U].rearrange("p (u h) -> p u h", h=H),
                    in1=recip[:][:, None, :].broadcast_to([P, UD, H]),
                    op=mybir.AluOpType.mult)
                if not nobias:
                    nc.vector.tensor_tensor(out=o1[:], in0=o1[:],
                                            in1=outb[:],
                                            op=mybir.AluOpType.add)
                nc.sync.dma_start(out_d[b * P: (b + 1) * P], o1[:])

        for t in range(NT0):
            emit_kv_tile(t)
        for b in range(NB):
            emit_block(b, 0)
            t = NT0 + b
            if t < NT:
                emit_kv_tile(t)
        for b in range(NB):
            emit_block(b, 1)


# ---------------------------------------------------------------- execution
class SpmdRunner:
    def __init__(self, nc, n_cores=C):
        import jax
        from jax.sharding import Mesh, PartitionSpec
        from jax.experimental.shard_map import shard_map
        from concourse.bass2jax import (_bass_exec_p, install_neuronx_cc_hook,
                                        partition_id_tensor)
        install_neuronx_cc_hook()
        self.jax = jax
        self.nc = nc
        self.n_cores = n_cores
        partition_name = nc.partition_id_tensor.name if nc.partition_id_tensor else None
        in_names, out_names, out_avals = [], [], []
        for alloc in nc.m.functions[0].allocations:
            if not isinstance(alloc, mybir.MemoryLocationSet):
                continue
            name = alloc.memorylocations[0].name
            if alloc.kind == "ExternalInput":
                if name != partition_name:
                    in_names.append(name)
            elif alloc.kind == "ExternalOutput":
                out_names.append(name)
                out_avals.append(jax.core.ShapedArray(
                    tuple(alloc.tensor_shape), mybir.dt.np(alloc.dtype)))
        self.in_names, self.out_names, self.out_avals = in_names, out_names, out_avals
        n_params = len(in_names)

        all_in_names = list(in_names) + list(out_names)
        if partition_name is not None:
            all_in_names.append(partition_name)

        def _body(*args):
            operands = list(args)
            if partition_name is not None:
                operands.append(partition_id_tensor())
            outs = _bass_exec_p.bind(
                *operands,
                out_avals=tuple(out_avals),
                in_names=tuple(all_in_names),
                out_names=tuple(out_names),
                lowering_input_output_aliases=(),
                sim_require_finite=False,
                sim_require_nnan=False,
                nc=nc,
            )
            return tuple(outs)

        devices = jax.devices()[:n_cores]
        self.mesh = Mesh(np.asarray(devices), ("core",))
        n_extra = len(out_names)
        in_specs = (PartitionSpec("core"),) * (n_params + n_extra)
        out_specs = (PartitionSpec("core"),) * len(out_names)
        self.fn = jax.jit(
            shard_map(_body, mesh=self.mesh, in_specs=in_specs,
                      out_specs=out_specs, check_rep=False),
            keep_unused=True,
        )

    def put_inputs(self, in_maps):
        from jax.sharding import NamedSharding, PartitionSpec
        sharding = NamedSharding(self.mesh, PartitionSpec("core"))
        args = []
        for name in self.in_names:
            concat = np.concatenate([np.asarray(m[name]) for m in in_maps], axis=0)
            args.append(self.jax.device_put(concat, sharding))
        for av in self.out_avals:
            args.append(self.jax.device_put(
                np.zeros((self.n_cores * av.shape[0], *av.shape[1:]), av.dtype),
                sharding))
        return args

    def __call__(self, args):
        outs = self.fn(*args)
        self.jax.block_until_ready(outs)
        return outs

    def run_to_numpy(self, args):
        outs = self(args)
        res = []
        for c in range(self.n_cores):
            d = {}
            for i, name in enumerate(self.out_names):
                d[name] = np.asarray(outs[i]).reshape(
                    self.n_cores, *self.out_avals[i].shape)[c]
            res.append(d)
        return res


_CACHE = {}


def _get_runner(CHH, reps=1, nobias=False):
    key = (CHH, reps, nobias)
    if key not in _CACHE:
        nc = build_program(CHH, reps=reps, nobias=nobias)
        _CACHE[key] = SpmdRunner(nc)
    return _CACHE[key]


def kernel(x, edge_index, query_kernel, query_bias, key_kernel, key_bias,
           kernel, bias):
    in_maps, CHH, perm, nobias = preprocess(x, edge_index, query_kernel,
                                            query_bias, key_kernel, key_bias,
                                            kernel, bias)
    runner = _get_runner(CHH, nobias=nobias)
    args = runner.put_inputs(in_maps)
    res = runner.run_to_numpy(args)
    # device output columns are head-minor (u' = ud*H + h); un-permute
    uperm = np.array([h * UD + ud for ud in range(UD) for h in range(H)])
    inv = np.argsort(uperm)
    out = np.empty((N, U), np.float32)
    for c in range(C):
        valid = perm[c] >= 0
        out[perm[c][valid]] = res[c]["out"][valid][:, inv]
    return out

